# revision 3
# baseline (speedup 1.0000x reference)
"""LDA loss on 8 trn2 cores — two launches, no collectives.

L1 (per core): chunked 2-queue DMA of fea shard in [p, c, d] f32 layout;
group-sum centers C_pd (DVE permuted reduce / GP pyramid); diff = x - c
(DVE tensor_tensor, slot broadcast); square (ACT); d2 (DVE, bf16 2x);
intra hinge -> ipart. PE transposes C -> sqrt2*C_dT bf16; sq.
Exports: cloc [128, 1024] bf16, sqloc [8, 128] f32, ipart [128, 1].

Host: circulant rotation per core; feeds L2.

L2 (per core): gram rows m'=0..7 (own 1024 centers) vs rotated cols
[128m', 128m'+4224); psum = 2g (+ -512 I diag); per-tile certificate:
  2p: DVE sub(sqb - psum) + min-reduce
  da: DVE sub -> bf16, ACT Relu(-x + 1 - sq_i) accum
  pa: PE K=1 adds -sq_j, ACT Relu(psum + 1 - sq_i) accum
Host: min d2 >= 1 and relu sums == 0 -> inter = 0.0 (numpy fallback else);
intra = sum(ipart)/B.
"""
import os
import sys

if "/opt/trn_rl_repo" not in sys.path:
    sys.path.insert(0, "/opt/trn_rl_repo")

import numpy as np
import ml_dtypes

import concourse.bacc as bacc
import concourse.tile as tile
from concourse import mybir
from concourse.bass_utils import run_bass_kernel_spmd

N_CORES = 8
B, D, P = 131072, 128, 16
G = B // P
GL = G // N_CORES
SL = B // N_CORES
BIG = 512.0
SQ2 = float(np.sqrt(2.0))

F32 = mybir.dt.float32
BF16 = mybir.dt.bfloat16

AF = mybir.ActivationFunctionType
OP = mybir.AluOpType
AX = mybir.AxisListType

DMA_SPLIT = os.environ.get("K_DMA_SPLIT", "1") == "1"
CRED_GP = set(int(x) for x in os.environ.get("K_CRED_GP", "3,4,5,6,7").split(",") if x != "")
SQ_DVE = set(int(x) for x in os.environ.get("K_SQ_DVE", "").split(",") if x != "")
DIFF_MODE = os.environ.get("K_DIFF", "zs")   # zs | cp
# L2 per-tile modes by tix (5 tiles per row): 2p | da | pa
TILE_MODES = os.environ.get("K_TMODES", "2p,pa,pa,da,2p").split(",")

_cache = {}
_last = {}


def _build_l1():
    nc = bacc.Bacc("TRN2", target_bir_lowering=False, debug=False,
                   num_devices=N_CORES)
    fea = nc.dram_tensor("fea", [SL, D], F32, kind="ExternalInput").ap()
    ident = nc.dram_tensor("ident", [128, 128], BF16, kind="ExternalInput").ap()
    identf = nc.dram_tensor("identf", [128, 128], F32, kind="ExternalInput").ap()

    ipart = nc.dram_tensor("ipart", [128, 1], F32, kind="ExternalOutput").ap()
    cloc = nc.dram_tensor("cloc", [128, 1024], BF16, kind="ExternalOutput").ap()
    sqloc = nc.dram_tensor("sqloc", [8, 128], F32, kind="ExternalOutput").ap()

    fea3 = fea.rearrange("(p c) d -> p c d", p=128)

    with tile.TileContext(nc) as tc:
        with (
            tc.tile_pool(name="pp", bufs=1) as pp,
            tc.tile_pool(name="xf", bufs=4) as xfp,
            tc.tile_pool(name="wk", bufs=2) as wp,
        ):
            t_id = pp.tile([128, 128], BF16, tag="id")
            nc.sync.dma_start(t_id[:], ident[:])
            t_idf = pp.tile([128, 128], F32, tag="idf")
            nc.sync.dma_start(t_idf[:], identf[:])

            t_cpd = pp.tile([128, 1024], F32, tag="cpd")
            t_cpdb = pp.tile([128, 1024], BF16, tag="cpdb")
            t_d2 = pp.tile([128, 128], F32, tag="d2")
            t_csq = pp.tile([128, 1024], BF16, tag="csq")
            pmid_ctx = tc.tile_pool(name="pmid", bufs=1, space="PSUM")
            pmid = pmid_ctx.__enter__()
            ps_tr = pmid.tile([128, 1024], BF16, tag="tr")

            def emit_load_cred(q):
                t_xf = xfp.tile([128, 2048], F32, tag="xf")
                if DMA_SPLIT:
                    xq = t_xf[:].rearrange("p (c d) -> p c d", d=128)
                    nc.sync.dma_start(xq[:, 0:8, :],
                                      fea3[:, 16 * q:16 * q + 8, :])
                    nc.scalar.dma_start(xq[:, 8:16, :],
                                        fea3[:, 16 * q + 8:16 * q + 16, :])
                else:
                    nc.sync.dma_start(
                        t_xf[:].rearrange("p (c d) -> p c d", d=128)[:, :, :],
                        fea3[:, 16 * q:16 * (q + 1), :])
                cq = t_cpd[:, 128 * q:128 * (q + 1)]
                eng2 = nc.gpsimd if q in CRED_GP else nc.vector
                t_s1 = wp.tile([128, 1024], F32, tag="gs1")
                eng2.tensor_tensor(t_s1[:], t_xf[:, 0:1024],
                                   t_xf[:, 1024:2048], op=OP.add)
                t_s2 = wp.tile([128, 512], F32, tag="gs2")
                eng2.tensor_tensor(t_s2[:], t_s1[:, 0:512],
                                   t_s1[:, 512:1024], op=OP.add)
                t_s3 = wp.tile([128, 256], F32, tag="gs3")
                eng2.tensor_tensor(t_s3[:], t_s2[:, 0:256],
                                   t_s2[:, 256:512], op=OP.add)
                eng2.tensor_tensor(cq, t_s3[:, 0:128], t_s3[:, 128:256],
                                   op=OP.add)
                nc.scalar.activation(t_cpdb[:, 128 * q:128 * (q + 1)],
                                     cq, AF.Copy, scale=1.0 / 16.0)
                nc.scalar.activation(t_csq[:, 128 * q:128 * (q + 1)],
                                     t_cpdb[:, 128 * q:128 * (q + 1)],
                                     AF.Square)
                nc.tensor.transpose(ps_tr[:, 128 * q:128 * (q + 1)],
                                    t_cpdb[:, 128 * q:128 * (q + 1)], t_id[:])
                return t_xf

            def emit_diff_d2(q, t_xf):
                xq = t_xf[:].rearrange("p (c d) -> p c d", d=128)
                t_df = wp.tile([128, 2048], BF16, tag="df")
                df3 = t_df[:].rearrange("p (c d) -> p c d", d=128)
                cb_b = t_cpdb[:, 128 * q:128 * (q + 1)].rearrange(
                    "p (o d) -> p o d", o=1).broadcast_to((128, 16, 128))
                nc.vector.tensor_tensor(df3[:, :, :], xq[:, :, :], cb_b,
                                        op=OP.subtract)
                t_ds = wp.tile([128, 2048], BF16, tag="dsq")
                if q in SQ_DVE:
                    nc.vector.tensor_tensor(t_ds[:], t_df[:], t_df[:],
                                            op=OP.mult)
                else:
                    nc.scalar.activation(t_ds[:], t_df[:], AF.Square)
                ds3 = t_ds[:].rearrange("p (c d) -> p c d", d=128)
                nc.vector.tensor_reduce(
                    t_d2[:, 16 * q:16 * (q + 1)], ds3, axis=AX.X, op=OP.add)

            pend = []
            for q in range(8):
                xf = emit_load_cred(q)
                pend.append((q, xf))
                if len(pend) > 1:
                    emit_diff_d2(*pend.pop(0))
            while pend:
                emit_diff_d2(*pend.pop(0))

            # intra tail
            t_di = pp.tile([128, 128], F32, tag="di")
            nc.scalar.activation(t_di[:], t_d2[:], AF.Sqrt)
            t_hw = pp.tile([128, 128], F32, tag="hw")
            nc.vector.tensor_scalar(t_hw[:], t_di[:], 0.1, 0.0,
                                    op0=OP.subtract, op1=OP.max)
            t_hsq = pp.tile([128, 128], F32, tag="hsq")
            t_ip = pp.tile([128, 1], F32, tag="ip")
            nc.scalar.activation(t_hsq[:], t_hw[:], AF.Square,
                                 accum_out=t_ip[:])
            nc.sync.dma_start(ipart[:], t_ip[:])

            # sq + exports (transposes/csq already emitted per chunk)
            t_sqpq = pp.tile([128, 8], F32, tag="sqpq")
            nc.vector.tensor_reduce(
                t_sqpq[:], t_csq[:].rearrange("p (q d) -> p q d", d=128),
                axis=AX.X, op=OP.add)
            ps_sq = pmid.tile([8, 128], F32, tag="sq8")
            nc.tensor.transpose(ps_sq[:], t_sqpq[:], t_idf[:])
            t_sq8 = pp.tile([8, 128], F32, tag="sq8s")
            nc.scalar.copy(t_sq8[:], ps_sq[:])
            nc.sync.dma_start(sqloc[:], t_sq8[:])
            t_cloc = pp.tile([128, 1024], BF16, tag="cloc")
            nc.scalar.activation(t_cloc[:], ps_tr[:], AF.Copy, scale=SQ2)
            nc.sync.dma_start(cloc[:], t_cloc[:])
            pmid_ctx.__exit__(None, None, None)

    nc.compile()
    return nc


def _build_l2():
    nc = bacc.Bacc("TRN2", target_bir_lowering=False, debug=False,
                   num_devices=N_CORES)
    call = nc.dram_tensor("call", [128, G], BF16, kind="ExternalInput").ap()
    sqr = nc.dram_tensor("sqr", [1, G], F32, kind="ExternalInput").ap()
    sqn = nc.dram_tensor("sqn", [1, G], BF16, kind="ExternalInput").ap()
    sqi = nc.dram_tensor("sqi", [128, 8], F32, kind="ExternalInput").ap()
    ident = nc.dram_tensor("ident", [128, 128], BF16, kind="ExternalInput").ap()
    idneg = nc.dram_tensor("idneg", [128, 128], BF16, kind="ExternalInput").ap()
    ones1 = nc.dram_tensor("ones1", [1, 128], BF16, kind="ExternalInput").ap()

    minrow = nc.dram_tensor("minrow", [128, 8], F32, kind="ExternalOutput").ap()
    relus = nc.dram_tensor("relus", [128, 1], F32, kind="ExternalOutput").ap()

    # 512-col psum tiles; per row mi: tiles between 512 boundaries over
    # [base, 4224+base); two waves (<=2048 / rest) so <=5 banks in flight
    # while matmuls of one wave share a single ldweights of lhs.
    def row_tiles(mi):
        base = 128 * mi
        out = []
        a = base
        while a < 4224 + base:
            b = min((a // 512 + 1) * 512, 4224 + base)
            out.append((a, b))
            a = b
        return out

    MODE_BY_IX = os.environ.get(
        "K_TMODES", "2p,pa,pa,pa,da,da,da,2p,2p,2p").split(",")

    with tile.TileContext(nc) as tc:
        with (
            tc.tile_pool(name="pp", bufs=1) as pp,
            tc.tile_pool(name="wk", bufs=3) as wp,
        ):
            t_call = pp.tile([128, G], BF16, tag="call")
            for k in range(8):
                eng = nc.scalar if (DMA_SPLIT and k % 2 == 1) else nc.sync
                eng.dma_start(t_call[:, 1024 * k:1024 * (k + 1)],
                              call[:, 1024 * k:1024 * (k + 1)])
            t_sqrr = pp.tile([1, G], F32, tag="sqr")
            nc.sync.dma_start(t_sqrr[:], sqr[:])
            t_sqn = pp.tile([1, G], BF16, tag="sqn")
            nc.sync.dma_start(t_sqn[:], sqn[:])
            t_sqi = pp.tile([128, 8], F32, tag="sqi")
            nc.sync.dma_start(t_sqi[:], sqi[:])
            t_id = pp.tile([128, 128], BF16, tag="id")
            nc.sync.dma_start(t_id[:], ident[:])
            t_idn = pp.tile([128, 128], BF16, tag="idn")
            nc.sync.dma_start(t_idn[:], idneg[:])
            t_o1 = pp.tile([1, 128], BF16, tag="o1")
            nc.sync.dma_start(t_o1[:], ones1[:])

            t_bias = pp.tile([128, 8], F32, tag="bias")
            nc.vector.tensor_scalar(t_bias[:], t_sqi[:], -1.0, 1.0,
                                    op0=OP.mult, op1=OP.add)

            t_sqb = []
            for blk in range(4):
                tb = pp.tile([128, 2048], F32, tag=f"sqb{blk}")
                nc.gpsimd.partition_broadcast(
                    tb[:], t_sqrr[0:1, 2048 * blk:2048 * (blk + 1)])
                t_sqb.append(tb)

            NSLOT = 10
            t_mins = pp.tile([128, 8 * NSLOT], F32, tag="mins")
            t_racc = pp.tile([128, 8 * NSLOT], F32, tag="racc")
            nc.gpsimd.memset(t_racc[:], 0.0)
            nc.gpsimd.memset(t_mins[:], 1e30)

            with tc.tile_pool(name="pg", bufs=8, space="PSUM") as pg:
                for mi in range(8):
                    base = 128 * mi
                    lhs = t_call[:, base:base + 128]
                    tiles = row_tiles(mi)
                    waves = [[(i, t) for i, t in enumerate(tiles)
                              if t[0] < 2048],
                             [(i, t) for i, t in enumerate(tiles)
                              if t[0] >= 2048]]
                    for wi, wave in enumerate(waves):
                        psts = {}
                        # gram matmuls back-to-back, one lhs load
                        for i, (lo, hi) in wave:
                            n = hi - lo
                            mode = MODE_BY_IX[i]
                            pst = pg.tile([128, 512], F32, tag="ps")
                            psts[i] = pst
                            more = (mode == "pa") or (lo <= base < hi)
                            nc.tensor.matmul(pst[:, 0:n], lhs,
                                             t_call[:, lo:hi],
                                             start=True, stop=not more,
                                             skip_group_check=True)
                        # sq-row adds for pa tiles
                        for i, (lo, hi) in wave:
                            if MODE_BY_IX[i] != "pa":
                                continue
                            n = hi - lo
                            nc.tensor.matmul(
                                psts[i][:, 0:n], t_o1[:], t_sqn[0:1, lo:hi],
                                start=False, stop=not (lo <= base < hi),
                                skip_group_check=True)
                        # diagonal fix (wave 0 only)
                        if wi == 0:
                            for i, (lo, hi) in wave:
                                if lo <= base < hi:
                                    off = base - lo
                                    nc.tensor.matmul(
                                        psts[i][:, off:off + 128],
                                        t_idn[:], t_id[:],
                                        start=False, stop=True,
                                        skip_group_check=True)
                        # consumers
                        for i, (lo, hi) in wave:
                            n = hi - lo
                            mode = MODE_BY_IX[i]
                            slot = NSLOT * mi + i
                            ps = psts[i][:, 0:n]
                            blk = lo // 2048
                            off = lo - 2048 * blk
                            if mode == "pa":
                                t_ro = wp.tile([128, 512], BF16, tag="ro")
                                nc.scalar.activation(
                                    t_ro[:, 0:n], ps, AF.Relu,
                                    bias=t_bias[:, mi:mi + 1], scale=1.0,
                                    accum_out=t_racc[:, slot:slot + 1])
                            elif mode == "da":
                                t_sc = wp.tile([128, 512], BF16, tag="sc")
                                nc.vector.tensor_tensor(
                                    t_sc[:, 0:n],
                                    t_sqb[blk][:, off:off + n], ps,
                                    op=OP.subtract)
                                t_ro = wp.tile([128, 512], BF16, tag="ro")
                                nc.scalar.activation(
                                    t_ro[:, 0:n], t_sc[:, 0:n], AF.Relu,
                                    bias=t_bias[:, mi:mi + 1], scale=-1.0,
                                    accum_out=t_racc[:, slot:slot + 1])
                            else:
                                t_sc = wp.tile([128, 512], BF16, tag="sc")
                                seng = nc.gpsimd if mode == "g2" else nc.vector
                                seng.tensor_tensor(
                                    t_sc[:, 0:n],
                                    t_sqb[blk][:, off:off + n], ps,
                                    op=OP.subtract)
                                nc.vector.tensor_reduce(
                                    t_mins[:, slot:slot + 1], t_sc[:, 0:n],
                                    axis=AX.X, op=OP.min)

            t_minc = pp.tile([128, 8], F32, tag="minc")
            nc.vector.tensor_reduce(
                t_minc[:], t_mins[:].rearrange("p (m t) -> p m t", t=NSLOT),
                axis=AX.X, op=OP.min)
            nc.sync.dma_start(minrow[:], t_minc[:])
            t_rsum = pp.tile([128, 1], F32, tag="rsum")
            nc.vector.tensor_reduce(t_rsum[:], t_racc[:],
                                    axis=AX.X, op=OP.add)
            nc.sync.dma_start(relus[:], t_rsum[:])

    nc.compile()
    return nc


def _get(name, builder):
    if name not in _cache:
        _cache[name] = builder()
    return _cache[name]


def _host_fallback(fea):
    f = fea.astype(np.float64).reshape(G, P, D)
    c = f.mean(1)
    sq = (c * c).sum(1)
    tot = 0.0
    for i in range(0, G, 512):
        blk = sq[i:i + 512, None] + sq[None, :] - 2.0 * (c[i:i + 512] @ c.T)
        d = np.sqrt(np.maximum(blk, 0.0))
        h = np.maximum(1.0 - d, 0.0) ** 2
        iu = np.triu(np.ones((512, G), dtype=bool), k=1 + i)
        tot += h[iu].sum()
    return np.float32(tot / (G * (G - 1) / 2.0))


def kernel(path_fea):
    fea = np.ascontiguousarray(
        np.asarray(path_fea, dtype=np.float32).reshape(B, D))

    trace = bool(int(os.environ.get("KERNEL_TRACE", "0")))
    runkw = {}
    if trace:
        import trace_shim
        trace_shim.install()
        runkw = dict(trace=True)

    nc1 = _get("l1", _build_l1)
    ident = np.eye(128, dtype=np.float32).astype(ml_dtypes.bfloat16)
    identf = np.eye(128, dtype=np.float32)
    ins1 = [{"fea": fea[SL * cc:SL * (cc + 1)], "ident": ident,
             "identf": identf} for cc in range(N_CORES)]
    r1 = run_bass_kernel_spmd(nc1, ins1, core_ids=list(range(N_CORES)),
                              **runkw)
    if trace and r1.exec_time_ns is not None:
        print(f"[launch1] HW exec time: {r1.exec_time_ns} ns")
        _last["r1"] = r1

    clocs = [r1.results[cc]["cloc"] for cc in range(N_CORES)]
    sq8s = [r1.results[cc]["sqloc"] for cc in range(N_CORES)]
    ipart_sum = sum(float(r1.results[cc]["ipart"].astype(np.float64).sum())
                    for cc in range(N_CORES))

    idneg = (-BIG * np.eye(128, dtype=np.float32)).astype(ml_dtypes.bfloat16)
    ones1 = np.ones((1, 128), np.float32).astype(ml_dtypes.bfloat16)
    ins2 = []
    for cc in range(N_CORES):
        order = [(cc + t) % 8 for t in range(8)]
        callm = np.concatenate([clocs[t] for t in order], axis=1)
        sqrow = np.concatenate(
            [sq8s[t].reshape(-1) for t in order]).reshape(1, G)
        sqn = (-sqrow).astype(ml_dtypes.bfloat16)
        sqi = np.ascontiguousarray(sq8s[cc].T)
        ins2.append({"call": np.ascontiguousarray(callm),
                     "sqr": sqrow.astype(np.float32), "sqn": sqn,
                     "sqi": sqi, "ident": ident, "idneg": idneg,
                     "ones1": ones1})

    nc2 = _get("l2", _build_l2)
    r2 = run_bass_kernel_spmd(nc2, ins2, core_ids=list(range(N_CORES)),
                              **runkw)
    if trace and r2.exec_time_ns is not None:
        print(f"[launch2] HW exec time: {r2.exec_time_ns} ns")
        _last["r2"] = r2

    min_d2 = np.inf
    relu_tot = 0.0
    for cc in range(N_CORES):
        res = r2.results[cc]
        mr = res["minrow"]
        sql = sq8s[cc]
        d2 = mr + sql.T
        min_d2 = min(min_d2, float(d2.min()))
        relu_tot += float(np.abs(res["relus"]).sum())

    if min_d2 >= 1.0 and relu_tot == 0.0:
        inter = np.float32(0.0)
    else:
        inter = _host_fallback(fea)
    intra = np.float32(ipart_sum / B)
    return (inter, intra)
